# revision 5
# baseline (speedup 1.0000x reference)
"""Trainium2 Bass kernel for nn_CrossAttention (B=2, Nq=Nk=2048, C=1024, H=16).

Sharding: 8 cores; core c -> batch b=c//4, query-slice r=c%4 (512 queries),
all 16 heads. K/V projections computed on the core's kv-slice (512 tokens),
then AllGathered (bf16) within each 4-core batch group. logits_mean and x
slices are written directly (no output collectives). aff_mask computed
redundantly per core from the gathered K.

All matmuls in bf16 (fp32 PSUM accumulation); measured end-to-end error vs
fp32 reference ~2-3e-3 scale-relative.
"""
import sys

for _p in ("/opt/trn_rl_repo", "/root/.axon_site", "/root/.axon_site/_ro/trn_rl_repo",
           "/root/.axon_site/_ro/pypackages"):
    if _p not in sys.path:
        sys.path.append(_p)

import numpy as np

B, T, C, H, Dh = 2, 2048, 1024, 16, 64
QS = 512          # queries per core
KS = 512          # kv tokens per core (pre-gather)
P = 128
NCORES = 8
GROUPS = [[0, 1, 2, 3], [4, 5, 6, 7]]
SC_ATTN = float((Dh ** -0.5) / 1.5)    # scale/ATTN_TEMP
SC_CLS = float((Dh ** -0.5) / 2.0)     # scale/CLS_TEMP
SC_LM = SC_ATTN / H                    # logits-mean output scale

_CACHE = {}


def _build_nc():
    import concourse.bass as bass
    from concourse import bacc
    import concourse.mybir as mybir
    import concourse.tile as tile
    from contextlib import ExitStack

    f32, bf16 = mybir.dt.float32, mybir.dt.bfloat16
    AF = mybir.ActivationFunctionType
    ALU = mybir.AluOpType

    nc = bacc.Bacc("TRN2", target_bir_lowering=False, debug=False, num_devices=NCORES)

    xq = nc.dram_tensor("xq", [QS, C], f32, kind="ExternalInput")
    xkv = nc.dram_tensor("xkv", [KS, C], f32, kind="ExternalInput")
    cls = nc.dram_tensor("cls", [C], f32, kind="ExternalInput")
    Wq = nc.dram_tensor("Wq", [C, C], f32, kind="ExternalInput")
    Wk = nc.dram_tensor("Wk", [C, C], f32, kind="ExternalInput")
    Wv = nc.dram_tensor("Wv", [C, C], f32, kind="ExternalInput")
    Wp = nc.dram_tensor("Wp", [C, C], f32, kind="ExternalInput")
    bq = nc.dram_tensor("bq", [C], f32, kind="ExternalInput")
    bk = nc.dram_tensor("bk", [C], f32, kind="ExternalInput")
    bv = nc.dram_tensor("bv", [C], f32, kind="ExternalInput")
    bp = nc.dram_tensor("bp", [C], f32, kind="ExternalInput")

    x_out = nc.dram_tensor("x_out", [QS, C], f32, kind="ExternalOutput")
    lm_out = nc.dram_tensor("lm_out", [QS, T], f32, kind="ExternalOutput")
    aff_out = nc.dram_tensor("aff_out", [T], f32, kind="ExternalOutput")

    KAG = 8 * P * KS          # kT contribution elems, flat order (s p f)
    VAG = KS * C              # v contribution elems, flat order (p tt o)

    with tile.TileContext(nc) as tc:
        with ExitStack() as ctx:
            const = ctx.enter_context(tc.tile_pool(name="const", bufs=1))
            work = ctx.enter_context(tc.tile_pool(name="work", bufs=2))
            dram = ctx.enter_context(tc.tile_pool(name="dram", bufs=1, space="DRAM"))
            ps_misc = ctx.enter_context(tc.tile_pool(name="ps_misc", bufs=2, space="PSUM"))
            ps_st = ctx.enter_context(tc.tile_pool(name="ps_st", bufs=2, space="PSUM"))
            ps_ctx = ctx.enter_context(tc.tile_pool(name="ps_ctx", bufs=2, space="PSUM"))

            # ---------- constants: biases, cls
            bq_sb = const.tile([P, 8], f32)
            nc.sync.dma_start(bq_sb[:], bq[:].rearrange("(s p) -> p s", p=P))
            bk_sb = const.tile([P, 8], f32)
            nc.sync.dma_start(bk_sb[:], bk[:].rearrange("(s p) -> p s", p=P))
            bv_rep = const.tile([P, C], f32)
            _bv = bv[:]
            nc.sync.dma_start(bv_rep[:], bass.AP(tensor=_bv.tensor, offset=_bv.offset,
                                                 ap=[[0, P]] + list(_bv.ap)))
            bp_rep = const.tile([P, C], f32)
            _bp = bp[:]
            nc.sync.dma_start(bp_rep[:], bass.AP(tensor=_bp.tensor, offset=_bp.offset,
                                                 ap=[[0, P]] + list(_bp.ap)))
            clsf = const.tile([P, 8], f32)
            nc.sync.dma_start(clsf[:], cls[:].rearrange("(j p) -> p j", p=P))
            cls_bf = const.tile([P, 8], bf16)
            nc.vector.tensor_copy(cls_bf[:], clsf[:])

            qT = const.tile([P, 8, QS], bf16)

            # ---------- phase 0: bf16 casts + DMA transposes
            with tc.tile_pool(name="prep", bufs=1) as prep:
                def load_cast_transpose(src, n_row_tiles, name, out_pool):
                    # src [n*128, C] fp32 -> SBUF [128, 8, n*128] bf16 (= src.T)
                    sd = dram.tile([n_row_tiles * P, C], bf16, tag=f"{name}_dram")
                    for t0 in range(0, n_row_tiles, 4):
                        nt = min(4, n_row_tiles - t0)
                        st = prep.tile([P, 4, C], f32, tag="ld_st")
                        nc.sync.dma_start(
                            st[:, 0:nt, :],
                            src[t0 * P:(t0 + nt) * P, :].rearrange("(t p) c -> p t c", p=P))
                        sb = prep.tile([P, 4, C], bf16, tag="ld_bf")
                        nc.vector.tensor_copy(sb[:, 0:nt, :], st[:, 0:nt, :])
                        nc.sync.dma_start(
                            sd[t0 * P:(t0 + nt) * P, :].rearrange("(t p) c -> p t c", p=P),
                            sb[:, 0:nt, :])
                    out = out_pool.tile([P, 8, n_row_tiles * P], bf16, tag=f"{name}_T")
                    for cs in range(8):
                        nc.sync.dma_start(out[:, cs, :], sd[:, cs * P:(cs + 1) * P],
                                          transpose=True)
                    return out

                xqT = load_cast_transpose(xq, QS // P, "xqT", prep)
                xkvT = load_cast_transpose(xkv, KS // P, "xkvT", prep)
                WkT = load_cast_transpose(Wk, 8, "WkT", prep)
                WvT = load_cast_transpose(Wv, 8, "WvT", prep)
                WqT = load_cast_transpose(Wq, 8, "WqT", prep)
                WpT2 = load_cast_transpose(Wp, 8, "WpT", const)

                # ---------- phase 1: K/V projections on kv-slice + AllGather
                agin = dram.tile([KAG + VAG], bf16)
                agout = dram.tile([4, KAG + VAG], bf16)

                for os_ in range(8):    # kT_loc block os_: flat (s p f)
                    ps = ps_misc.tile([P, KS], f32, tag="proj")
                    for cs in range(8):
                        nc.tensor.matmul(ps[:], WkT[:, cs, os_ * P:(os_ + 1) * P],
                                         xkvT[:, cs, :], start=(cs == 0), stop=(cs == 7))
                    kt_loc = work.tile([P, KS], bf16, tag="ktloc")
                    nc.vector.tensor_scalar_add(kt_loc[:], ps[:], bk_sb[:, os_:os_ + 1])
                    nc.sync.dma_start(
                        agin[os_ * (P * KS):(os_ + 1) * (P * KS)]
                            .rearrange("(p f) -> p f", p=P),
                        kt_loc[:])

                for tt in range(4):     # v_loc: flat (p tt o)
                    for ob in range(2):
                        ps = ps_misc.tile([P, 512], f32, tag="proj")
                        for cs in range(8):
                            nc.tensor.matmul(ps[:], xkvT[:, cs, tt * P:(tt + 1) * P],
                                             WvT[:, cs, ob * 512:(ob + 1) * 512],
                                             start=(cs == 0), stop=(cs == 7))
                        v_loc = work.tile([P, 512], bf16, tag="vloc")
                        nc.vector.tensor_tensor(v_loc[:], ps[:],
                                                bv_rep[:, ob * 512:(ob + 1) * 512],
                                                ALU.add)
                        nc.sync.dma_start(
                            agin[KAG:]
                                .rearrange("(p tt o) -> p tt o", p=P, tt=4)
                                [:, tt, ob * 512:(ob + 1) * 512],
                            v_loc[:])

                nc.gpsimd.collective_compute(
                    "AllGather", ALU.bypass,
                    replica_groups=GROUPS,
                    ins=[agin[:].opt()], outs=[agout[:].opt()])

                # ---------- phase 2: Q projection (all heads, q-slice)
                for os_ in range(8):
                    ps = ps_misc.tile([P, QS], f32, tag="proj")
                    for cs in range(8):
                        nc.tensor.matmul(ps[:], WqT[:, cs, os_ * P:(os_ + 1) * P],
                                         xqT[:, cs, :], start=(cs == 0), stop=(cs == 7))
                    nc.vector.tensor_scalar_add(qT[:, os_, :], ps[:], bq_sb[:, os_:os_ + 1])

            # ---------- phase 3+: attention-phase pool (opens after prep frees)
            att = ctx.enter_context(tc.tile_pool(name="att", bufs=1))
            kT = att.tile([P, 8, 4, KS], bf16)    # [c-part, os, rank, tok]
            vtmp = att.tile([P, 4, 4, C], bf16)   # [tok-part, rank, tt, o]
            vaug = att.tile([P, 16, H, 65], bf16)
            maskT = att.tile([P, 16], f32)
            affT = att.tile([P, 16], f32)
            ctxu = att.tile([64, H, QS], bf16)
            ctxT = att.tile([P, 8, QS], bf16)
            den_pk = att.tile([P, 64], f32)
            rec_pk = att.tile([P, 64], f32)

            for r_ in range(4):
                nc.sync.dma_start(
                    kT[:, :, r_, :],
                    agout[r_, 0:KAG].rearrange("(s p f) -> p s f", s=8, p=P))
                nc.sync.dma_start(
                    vtmp[:, r_, :, :],
                    agout[r_, KAG:].rearrange("(p tt o) -> p tt o", p=P, tt=4))

            # ---------- phase 4: aff mask
            for kt in range(16):
                r_, f0 = kt // 4, (kt % 4) * P
                ps = ps_misc.tile([P, 512], f32, tag="proj")
                for h in range(H):
                    bp_ = 64 * (h % 2)
                    nc.tensor.matmul(ps[:, h:h + 1],
                                     kT[bp_:bp_ + 64, h // 2, r_, f0:f0 + P],
                                     cls_bf[bp_:bp_ + 64, h // 2:h // 2 + 1],
                                     start=True, stop=True,
                                     tile_position=(bp_, 0))
                sg = work.tile([P, 16], f32, tag="sg")
                nc.scalar.activation(sg[:], ps[:, 0:16], AF.Sigmoid, scale=SC_CLS)
                nc.vector.tensor_reduce(affT[:, kt:kt + 1], sg[:],
                                        axis=mybir.AxisListType.X, op=ALU.add)
            affdram = dram.tile([T], f32)
            nc.sync.dma_start(affdram[:].rearrange("(kt p) -> p kt", p=P), affT[:])
            afflin = const.tile([1, T], f32)
            nc.sync.dma_start(afflin[:], affdram[:].rearrange("(o f) -> o f", o=1))
            nc.vector.tensor_scalar_mul(afflin[:], afflin[:], 1.0 / H)
            mmin = work.tile([1, 1], f32, tag="mmin")
            nc.vector.tensor_reduce(mmin[:], afflin[:], axis=mybir.AxisListType.X,
                                    op=ALU.min)
            mmax = work.tile([1, 1], f32, tag="mmax")
            nc.vector.tensor_reduce(mmax[:], afflin[:], axis=mybir.AxisListType.X,
                                    op=ALU.max)
            rng = work.tile([1, 1], f32, tag="rng")
            nc.vector.tensor_tensor(rng[:], mmax[:], mmin[:], ALU.subtract)
            nc.vector.tensor_scalar_add(rng[:], rng[:], 1e-6)
            rcp = work.tile([1, 1], f32, tag="rcp")
            nc.vector.reciprocal(rcp[:], rng[:])
            mask_lin = const.tile([1, T], f32)
            nc.vector.tensor_tensor(mask_lin[:], afflin[:], mmin[:].to_broadcast((1, T)),
                                    ALU.subtract)
            nc.vector.tensor_tensor(mask_lin[:], mask_lin[:], rcp[:].to_broadcast((1, T)),
                                    ALU.mult)
            nc.sync.dma_start(aff_out[:].rearrange("(o f) -> o f", o=1), mask_lin[:])
            maskdram = dram.tile([T], f32)
            nc.sync.dma_start(maskdram[:].rearrange("(o f) -> o f", o=1), mask_lin[:])
            nc.sync.dma_start(maskT[:], maskdram[:].rearrange("(kt p) -> p kt", p=P))

            # ---------- phase 5: vaug [128, kt, h, 65] = [aff*v | ones]
            nc.vector.memset(vaug[:], 1.0)
            for kt in range(16):
                r_, tt = kt // 4, kt % 4
                nc.vector.tensor_scalar_mul(
                    vaug[:, kt, :, 0:64],
                    vtmp[:, r_, tt, :].rearrange("p (h d) -> p h d", h=H),
                    maskT[:, kt:kt + 1])

            # ---------- phase 6: attention (8 head pairs)
            dden = dram.tile([H, QS], f32)
            for hp in range(8):
                h0, h1 = 2 * hp, 2 * hp + 1
                cps0 = ps_ctx.tile([65, QS], f32, tag="ctx")
                cps1 = ps_ctx.tile([65, QS], f32, tag="ctx")
                for kt in range(16):
                    r_, f0 = kt // 4, (kt % 4) * P
                    st = ps_st.tile([P, 2 * QS], f32, tag="st")
                    nc.tensor.matmul(st[:, 0:QS],
                                     kT[0:64, hp, r_, f0:f0 + P], qT[0:64, hp, :],
                                     start=True, stop=True, tile_position=(0, 0))
                    nc.tensor.matmul(st[:, QS:2 * QS],
                                     kT[64:128, hp, r_, f0:f0 + P], qT[64:128, hp, :],
                                     start=True, stop=True, tile_position=(64, 0))
                    ex = work.tile([P, 2 * QS], bf16, tag="ex")
                    nc.scalar.activation(ex[:], st[:], AF.Exp, scale=SC_ATTN)
                    nc.tensor.matmul(cps0[:], vaug[:, kt, h0, :], ex[:, 0:QS],
                                     start=(kt == 0), stop=(kt == 15))
                    nc.tensor.matmul(cps1[:], vaug[:, kt, h1, :], ex[:, QS:2 * QS],
                                     start=(kt == 0), stop=(kt == 15))
                for h, cps in ((h0, cps0), (h1, cps1)):
                    nc.vector.tensor_copy(ctxu[:, h, :], cps[0:64, :])
                    dn = work.tile([P, QS], f32, tag="dn")
                    nc.vector.tensor_copy(dn[64:65, :], cps[64:65, :])
                    nc.sync.dma_start(dden[h:h + 1, :], dn[64:65, :])

            # packed reciprocal of all 16x512 denominators
            nc.sync.dma_start(den_pk[:], dden[:].rearrange("h (p f) -> (h p) f", p=8))
            nc.vector.reciprocal(rec_pk[:], den_pk[:])
            rdram = dram.tile([H * QS], f32)
            nc.sync.dma_start(rdram[:].rearrange("(q f) -> q f", q=P), rec_pk[:])

            # normalize + assemble ctxT [128, cs, q]
            for h in range(H):
                rrep = work.tile([64, QS], f32, tag="rrep")
                src = rdram[h * QS:(h + 1) * QS]
                nc.sync.dma_start(rrep[:], bass.AP(tensor=src.tensor, offset=src.offset,
                                                   ap=[[0, 64]] + list(src.ap)))
                cn = work.tile([64, QS], bf16, tag="cn")
                nc.vector.tensor_tensor(cn[:], ctxu[:, h, :], rrep[:], ALU.mult)
                bp_ = 64 * (h % 2)
                nc.sync.dma_start(ctxT[bp_:bp_ + 64, h // 2, :], cn[:])

            # ---------- phase 7: logits_mean
            for qt in range(4):
                for kb in range(4):
                    ps = ps_misc.tile([P, 512], f32, tag="proj")
                    for cs in range(8):
                        nc.tensor.matmul(ps[:], qT[:, cs, qt * P:(qt + 1) * P],
                                         kT[:, cs, kb, :],
                                         start=(cs == 0), stop=(cs == 7))
                    lm_sb = work.tile([P, 512], f32, tag="lm")
                    nc.vector.tensor_scalar_mul(lm_sb[:], ps[:], SC_LM)
                    nc.sync.dma_start(lm_out[qt * P:(qt + 1) * P, kb * 512:(kb + 1) * 512],
                                      lm_sb[:])

            # ---------- phase 8: out-projection
            for qt in range(4):
                for ob in range(2):
                    ps = ps_misc.tile([P, 512], f32, tag="proj")
                    for cs in range(8):
                        nc.tensor.matmul(ps[:], ctxT[:, cs, qt * P:(qt + 1) * P],
                                         WpT2[:, cs, ob * 512:(ob + 1) * 512],
                                         start=(cs == 0), stop=(cs == 7))
                    x_sb = work.tile([P, 512], f32, tag="x")
                    nc.vector.tensor_tensor(x_sb[:], ps[:],
                                            bp_rep[:, ob * 512:(ob + 1) * 512],
                                            ALU.add)
                    nc.sync.dma_start(x_out[qt * P:(qt + 1) * P, ob * 512:(ob + 1) * 512],
                                      x_sb[:])

    nc.compile()
    return nc


def _get_runner():
    if "runner" in _CACHE:
        return _CACHE["runner"]
    import jax
    import concourse.mybir as mybir
    from concourse.bass2jax import _bass_exec_p, install_neuronx_cc_hook, partition_id_tensor
    from jax.sharding import Mesh, PartitionSpec, NamedSharding
    from jax.experimental.shard_map import shard_map

    nc = _build_nc()
    install_neuronx_cc_hook()
    partition_name = nc.partition_id_tensor.name if nc.partition_id_tensor else None
    in_names, out_names, out_avals, zero_shapes = [], [], [], []
    for alloc in nc.m.functions[0].allocations:
        if not isinstance(alloc, mybir.MemoryLocationSet):
            continue
        name = alloc.memorylocations[0].name
        if alloc.kind == "ExternalInput":
            if name != partition_name:
                in_names.append(name)
        elif alloc.kind == "ExternalOutput":
            out_names.append(name)
            shape = tuple(alloc.tensor_shape)
            dtype = mybir.dt.np(alloc.dtype)
            out_avals.append(jax.core.ShapedArray(shape, dtype))
            zero_shapes.append((shape, dtype))
    n_params = len(in_names)
    all_in_names = list(in_names) + list(out_names)
    if partition_name is not None:
        all_in_names.append(partition_name)

    def _body(*args):
        operands = list(args)
        if partition_name is not None:
            operands.append(partition_id_tensor())
        outs = _bass_exec_p.bind(
            *operands,
            out_avals=tuple(out_avals),
            in_names=tuple(all_in_names),
            out_names=tuple(out_names),
            lowering_input_output_aliases=(),
            sim_require_finite=True,
            sim_require_nnan=True,
            nc=nc,
        )
        return tuple(outs)

    devices = jax.devices()[:NCORES]
    mesh = Mesh(np.asarray(devices), ("core",))
    specs = (PartitionSpec("core"),)
    fn = jax.jit(
        shard_map(_body, mesh=mesh,
                  in_specs=specs * (n_params + len(out_names)),
                  out_specs=specs * len(out_names), check_rep=False),
        keep_unused=True,
    )
    sharding = NamedSharding(mesh, PartitionSpec("core"))
    _CACHE["runner"] = (fn, in_names, out_names, out_avals, zero_shapes, sharding)
    return _CACHE["runner"]


def kernel(x_q, x_kv, cls_token, Wq, bq, Wk, bk, Wv, bv, Wp, bp):
    fn, in_names, out_names, out_avals, zero_shapes, sharding = _get_runner()
    import jax

    x_q = np.asarray(x_q, dtype=np.float32)
    x_kv = np.asarray(x_kv, dtype=np.float32)
    cls_token = np.asarray(cls_token, dtype=np.float32)
    weights = {"Wq": np.asarray(Wq, np.float32), "Wk": np.asarray(Wk, np.float32),
               "Wv": np.asarray(Wv, np.float32), "Wp": np.asarray(Wp, np.float32),
               "bq": np.asarray(bq, np.float32), "bk": np.asarray(bk, np.float32),
               "bv": np.asarray(bv, np.float32), "bp": np.asarray(bp, np.float32)}

    in_maps = []
    for c in range(NCORES):
        b, r = c // 4, c % 4
        m = {"xq": np.ascontiguousarray(x_q[b, r * QS:(r + 1) * QS]),
             "xkv": np.ascontiguousarray(x_kv[b, r * KS:(r + 1) * KS]),
             "cls": np.ascontiguousarray(cls_token[b, 0])}
        m.update(weights)
        in_maps.append(m)

    concat_in = [np.concatenate([in_maps[c][nm] for c in range(NCORES)], axis=0)
                 for nm in in_names]
    concat_zeros = [np.zeros((NCORES * s[0], *s[1:]), d) for (s, d) in zero_shapes]
    dev_in = [jax.device_put(a, sharding) for a in concat_in]
    dev_zeros = [jax.device_put(a, sharding) for a in concat_zeros]
    out = fn(*dev_in, *dev_zeros)
    res = [{nm: np.asarray(out[i]).reshape(NCORES, *out_avals[i].shape)[c]
            for i, nm in enumerate(out_names)}
           for c in range(NCORES)]

    x = np.empty((B, T, C), np.float32)
    lm = np.empty((B, T, T), np.float32)
    aff = np.empty((B, 1, 1, T), np.float32)
    for c in range(NCORES):
        b, r = c // 4, c % 4
        x[b, r * QS:(r + 1) * QS] = res[c]["x_out"]
        lm[b, r * QS:(r + 1) * QS] = res[c]["lm_out"]
    aff[0, 0, 0] = res[0]["aff_out"]
    aff[1, 0, 0] = res[4]["aff_out"]
    return (x, lm, aff)


# revision 6
# speedup vs baseline: 8.2526x; 8.2526x over previous
"""Trainium2 Bass kernel for nn_CrossAttention (B=2, Nq=Nk=2048, C=1024, H=16).

Sharding: 8 cores; core c -> batch b=c//4, query-slice r=c%4 (512 queries),
all 16 heads. K/V projections computed on the core's kv-slice (512 tokens),
then AllGathered (bf16) within each 4-core batch group. logits_mean and x
slices are written directly (no output collectives). aff_mask computed
redundantly per core from the gathered K.

All matmuls in bf16 (fp32 PSUM accumulation); measured end-to-end error vs
fp32 reference ~2-3e-3 scale-relative.
"""
import sys

for _p in ("/opt/trn_rl_repo", "/root/.axon_site", "/root/.axon_site/_ro/trn_rl_repo",
           "/root/.axon_site/_ro/pypackages"):
    if _p not in sys.path:
        sys.path.append(_p)

import numpy as np

B, T, C, H, Dh = 2, 2048, 1024, 16, 64
QS = 512          # queries per core
KS = 512          # kv tokens per core (pre-gather)
P = 128
NCORES = 8
GROUPS = [[0, 1, 2, 3], [4, 5, 6, 7]]
SC_ATTN = float((Dh ** -0.5) / 1.5)    # scale/ATTN_TEMP
SC_CLS = float((Dh ** -0.5) / 2.0)     # scale/CLS_TEMP
SC_LM = SC_ATTN / H                    # logits-mean output scale

_CACHE = {}


def _build_nc(repeat=1):
    import concourse.bass as bass
    from concourse import bacc
    import concourse.mybir as mybir
    import concourse.tile as tile
    from contextlib import ExitStack

    f32, bf16 = mybir.dt.float32, mybir.dt.bfloat16
    AF = mybir.ActivationFunctionType
    ALU = mybir.AluOpType

    nc = bacc.Bacc("TRN2", target_bir_lowering=False, debug=False, num_devices=NCORES)

    xq = nc.dram_tensor("xq", [QS, C], f32, kind="ExternalInput")
    xkv = nc.dram_tensor("xkv", [KS, C], f32, kind="ExternalInput")
    cls = nc.dram_tensor("cls", [C], f32, kind="ExternalInput")
    Wq = nc.dram_tensor("Wq", [C, C], f32, kind="ExternalInput")
    Wk = nc.dram_tensor("Wk", [C, C], f32, kind="ExternalInput")
    Wv = nc.dram_tensor("Wv", [C, C], f32, kind="ExternalInput")
    Wp = nc.dram_tensor("Wp", [C, C], f32, kind="ExternalInput")
    bq = nc.dram_tensor("bq", [C], f32, kind="ExternalInput")
    bk = nc.dram_tensor("bk", [C], f32, kind="ExternalInput")
    bv = nc.dram_tensor("bv", [C], f32, kind="ExternalInput")
    bp = nc.dram_tensor("bp", [C], f32, kind="ExternalInput")

    x_out = nc.dram_tensor("x_out", [QS, C], f32, kind="ExternalOutput")
    lm_out = nc.dram_tensor("lm_out", [QS, T], f32, kind="ExternalOutput")
    aff_out = nc.dram_tensor("aff_out", [T], f32, kind="ExternalOutput")

    KAG = 8 * P * KS          # kT contribution elems, flat order (s p f)
    VAG = KS * C              # v contribution elems, flat order (p tt o)

    with tile.TileContext(nc) as tc:
      for _rep in range(repeat):
        with ExitStack() as ctx:
            const = ctx.enter_context(tc.tile_pool(name=f"const{_rep}", bufs=1))
            work = ctx.enter_context(tc.tile_pool(name=f"work{_rep}", bufs=2))
            dram = ctx.enter_context(tc.tile_pool(name=f"dram{_rep}", bufs=1, space="DRAM"))
            ps_misc = ctx.enter_context(tc.tile_pool(name=f"ps_misc{_rep}", bufs=2, space="PSUM"))
            ps_st = ctx.enter_context(tc.tile_pool(name=f"ps_st{_rep}", bufs=2, space="PSUM"))
            ps_ctx = ctx.enter_context(tc.tile_pool(name=f"ps_ctx{_rep}", bufs=2, space="PSUM"))

            # ---------- constants: biases, cls
            bq_sb = const.tile([P, 8], f32)
            nc.sync.dma_start(bq_sb[:], bq[:].rearrange("(s p) -> p s", p=P))
            bk_sb = const.tile([P, 8], f32)
            nc.sync.dma_start(bk_sb[:], bk[:].rearrange("(s p) -> p s", p=P))
            bv_rep = const.tile([P, C], f32)
            _bv = bv[:]
            nc.sync.dma_start(bv_rep[:], bass.AP(tensor=_bv.tensor, offset=_bv.offset,
                                                 ap=[[0, P]] + list(_bv.ap)))
            bp_rep = const.tile([P, C], f32)
            _bp = bp[:]
            nc.sync.dma_start(bp_rep[:], bass.AP(tensor=_bp.tensor, offset=_bp.offset,
                                                 ap=[[0, P]] + list(_bp.ap)))
            clsf = const.tile([P, 8], f32)
            nc.sync.dma_start(clsf[:], cls[:].rearrange("(j p) -> p j", p=P))
            cls_bf = const.tile([P, 8], bf16)
            nc.vector.tensor_copy(cls_bf[:], clsf[:])

            qT = const.tile([P, 8, QS], bf16)

            # ---------- phase 0: bf16 casts + DMA transposes
            with tc.tile_pool(name=f"prep{_rep}", bufs=1) as prep:
                def load_cast_transpose(src, n_row_tiles, name, out_pool):
                    # src [n*128, C] fp32 -> SBUF [128, 8, n*128] bf16 (= src.T)
                    sd = dram.tile([n_row_tiles * P, C], bf16, tag=f"{name}_dram")
                    for t0 in range(0, n_row_tiles, 4):
                        nt = min(4, n_row_tiles - t0)
                        st = prep.tile([P, 4, C], f32, tag="ld_st")
                        nc.sync.dma_start(
                            st[:, 0:nt, :],
                            src[t0 * P:(t0 + nt) * P, :].rearrange("(t p) c -> p t c", p=P))
                        sb = prep.tile([P, 4, C], bf16, tag="ld_bf")
                        nc.vector.tensor_copy(sb[:, 0:nt, :], st[:, 0:nt, :])
                        nc.sync.dma_start(
                            sd[t0 * P:(t0 + nt) * P, :].rearrange("(t p) c -> p t c", p=P),
                            sb[:, 0:nt, :])
                    out = out_pool.tile([P, 8, n_row_tiles * P], bf16, tag=f"{name}_T")
                    for cs in range(8):
                        nc.sync.dma_start(out[:, cs, :], sd[:, cs * P:(cs + 1) * P],
                                          transpose=True)
                    return out

                xqT = load_cast_transpose(xq, QS // P, "xqT", prep)
                xkvT = load_cast_transpose(xkv, KS // P, "xkvT", prep)
                WkT = load_cast_transpose(Wk, 8, "WkT", prep)
                WvT = load_cast_transpose(Wv, 8, "WvT", prep)
                WqT = load_cast_transpose(Wq, 8, "WqT", prep)
                WpT2 = load_cast_transpose(Wp, 8, "WpT", const)

                # ---------- phase 1: K/V projections on kv-slice + AllGather
                agin = dram.tile([KAG + VAG], bf16)
                agout = dram.tile([4, KAG + VAG], bf16)

                for os_ in range(8):    # kT_loc block os_: flat (s p f)
                    ps = ps_misc.tile([P, KS], f32, tag="proj")
                    for cs in range(8):
                        nc.tensor.matmul(ps[:], WkT[:, cs, os_ * P:(os_ + 1) * P],
                                         xkvT[:, cs, :], start=(cs == 0), stop=(cs == 7))
                    kt_loc = work.tile([P, KS], bf16, tag="ktloc")
                    nc.vector.tensor_scalar_add(kt_loc[:], ps[:], bk_sb[:, os_:os_ + 1])
                    nc.sync.dma_start(
                        agin[os_ * (P * KS):(os_ + 1) * (P * KS)]
                            .rearrange("(p f) -> p f", p=P),
                        kt_loc[:])

                for tt in range(4):     # v_loc: flat (p tt o)
                    for ob in range(2):
                        ps = ps_misc.tile([P, 512], f32, tag="proj")
                        for cs in range(8):
                            nc.tensor.matmul(ps[:], xkvT[:, cs, tt * P:(tt + 1) * P],
                                             WvT[:, cs, ob * 512:(ob + 1) * 512],
                                             start=(cs == 0), stop=(cs == 7))
                        v_loc = work.tile([P, 512], bf16, tag="vloc")
                        nc.vector.tensor_tensor(v_loc[:], ps[:],
                                                bv_rep[:, ob * 512:(ob + 1) * 512],
                                                ALU.add)
                        nc.sync.dma_start(
                            agin[KAG:]
                                .rearrange("(p tt o) -> p tt o", p=P, tt=4)
                                [:, tt, ob * 512:(ob + 1) * 512],
                            v_loc[:])

                nc.gpsimd.collective_compute(
                    "AllGather", ALU.bypass,
                    replica_groups=GROUPS,
                    ins=[agin[:].opt()], outs=[agout[:].opt()])

                # ---------- phase 2: Q projection (all heads, q-slice)
                for os_ in range(8):
                    ps = ps_misc.tile([P, QS], f32, tag="proj")
                    for cs in range(8):
                        nc.tensor.matmul(ps[:], WqT[:, cs, os_ * P:(os_ + 1) * P],
                                         xqT[:, cs, :], start=(cs == 0), stop=(cs == 7))
                    nc.vector.tensor_scalar_add(qT[:, os_, :], ps[:], bq_sb[:, os_:os_ + 1])

            # ---------- phase 3+: attention-phase pool (opens after prep frees)
            att = ctx.enter_context(tc.tile_pool(name=f"att{_rep}", bufs=1))
            kT = att.tile([P, 8, 4, KS], bf16)    # [c-part, os, rank, tok]
            vtmp = att.tile([P, 4, 4, C], bf16)   # [tok-part, rank, tt, o]
            vaug = att.tile([P, 16, H, 65], bf16)
            maskT = att.tile([P, 16], f32)
            affT = att.tile([P, 16], f32)
            ctxu = att.tile([64, H, QS], bf16)
            ctxT = att.tile([P, 8, QS], bf16)
            den_pk = att.tile([P, 64], f32)
            rec_pk = att.tile([P, 64], f32)

            for r_ in range(4):
                nc.sync.dma_start(
                    kT[:, :, r_, :],
                    agout[r_, 0:KAG].rearrange("(s p f) -> p s f", s=8, p=P))
                nc.sync.dma_start(
                    vtmp[:, r_, :, :],
                    agout[r_, KAG:].rearrange("(p tt o) -> p tt o", p=P, tt=4))

            # ---------- phase 4: aff mask
            for kt in range(16):
                r_, f0 = kt // 4, (kt % 4) * P
                ps = ps_misc.tile([P, 512], f32, tag="proj")
                for h in range(H):
                    bp_ = 64 * (h % 2)
                    nc.tensor.matmul(ps[:, h:h + 1],
                                     kT[bp_:bp_ + 64, h // 2, r_, f0:f0 + P],
                                     cls_bf[bp_:bp_ + 64, h // 2:h // 2 + 1],
                                     start=True, stop=True,
                                     tile_position=(bp_, 0))
                sg = work.tile([P, 16], f32, tag="sg")
                nc.scalar.activation(sg[:], ps[:, 0:16], AF.Sigmoid, scale=SC_CLS)
                nc.vector.tensor_reduce(affT[:, kt:kt + 1], sg[:],
                                        axis=mybir.AxisListType.X, op=ALU.add)
            affdram = dram.tile([T], f32)
            nc.sync.dma_start(affdram[:].rearrange("(kt p) -> p kt", p=P), affT[:])
            afflin = const.tile([1, T], f32)
            nc.sync.dma_start(afflin[:], affdram[:].rearrange("(o f) -> o f", o=1))
            nc.vector.tensor_scalar_mul(afflin[:], afflin[:], 1.0 / H)
            mmin = work.tile([1, 1], f32, tag="mmin")
            nc.vector.tensor_reduce(mmin[:], afflin[:], axis=mybir.AxisListType.X,
                                    op=ALU.min)
            mmax = work.tile([1, 1], f32, tag="mmax")
            nc.vector.tensor_reduce(mmax[:], afflin[:], axis=mybir.AxisListType.X,
                                    op=ALU.max)
            rng = work.tile([1, 1], f32, tag="rng")
            nc.vector.tensor_tensor(rng[:], mmax[:], mmin[:], ALU.subtract)
            nc.vector.tensor_scalar_add(rng[:], rng[:], 1e-6)
            rcp = work.tile([1, 1], f32, tag="rcp")
            nc.vector.reciprocal(rcp[:], rng[:])
            mask_lin = const.tile([1, T], f32)
            nc.vector.tensor_tensor(mask_lin[:], afflin[:], mmin[:].to_broadcast((1, T)),
                                    ALU.subtract)
            nc.vector.tensor_tensor(mask_lin[:], mask_lin[:], rcp[:].to_broadcast((1, T)),
                                    ALU.mult)
            nc.sync.dma_start(aff_out[:].rearrange("(o f) -> o f", o=1), mask_lin[:])
            maskdram = dram.tile([T], f32)
            nc.sync.dma_start(maskdram[:].rearrange("(o f) -> o f", o=1), mask_lin[:])
            nc.sync.dma_start(maskT[:], maskdram[:].rearrange("(kt p) -> p kt", p=P))

            # ---------- phase 5: vaug [128, kt, h, 65] = [aff*v | ones]
            nc.vector.memset(vaug[:], 1.0)
            for kt in range(16):
                r_, tt = kt // 4, kt % 4
                nc.vector.tensor_scalar_mul(
                    vaug[:, kt, :, 0:64],
                    vtmp[:, r_, tt, :].rearrange("p (h d) -> p h d", h=H),
                    maskT[:, kt:kt + 1])

            # ---------- phase 6: attention (8 head pairs)
            dden = dram.tile([H, QS], f32)
            for hp in range(8):
                h0, h1 = 2 * hp, 2 * hp + 1
                cps0 = ps_ctx.tile([65, QS], f32, tag="ctx")
                cps1 = ps_ctx.tile([65, QS], f32, tag="ctx")
                for kt in range(16):
                    r_, f0 = kt // 4, (kt % 4) * P
                    st = ps_st.tile([P, 2 * QS], f32, tag="st")
                    nc.tensor.matmul(st[:, 0:QS],
                                     kT[0:64, hp, r_, f0:f0 + P], qT[0:64, hp, :],
                                     start=True, stop=True, tile_position=(0, 0))
                    nc.tensor.matmul(st[:, QS:2 * QS],
                                     kT[64:128, hp, r_, f0:f0 + P], qT[64:128, hp, :],
                                     start=True, stop=True, tile_position=(64, 0))
                    ex = work.tile([P, 2 * QS], bf16, tag="ex")
                    nc.scalar.activation(ex[:], st[:], AF.Exp, scale=SC_ATTN)
                    nc.tensor.matmul(cps0[:], vaug[:, kt, h0, :], ex[:, 0:QS],
                                     start=(kt == 0), stop=(kt == 15))
                    nc.tensor.matmul(cps1[:], vaug[:, kt, h1, :], ex[:, QS:2 * QS],
                                     start=(kt == 0), stop=(kt == 15))
                for h, cps in ((h0, cps0), (h1, cps1)):
                    nc.vector.tensor_copy(ctxu[:, h, :], cps[0:64, :])
                    dn = work.tile([P, QS], f32, tag="dn")
                    nc.vector.tensor_copy(dn[64:65, :], cps[64:65, :])
                    nc.sync.dma_start(dden[h:h + 1, :], dn[64:65, :])

            # packed reciprocal of all 16x512 denominators
            nc.sync.dma_start(den_pk[:], dden[:].rearrange("h (p f) -> (h p) f", p=8))
            nc.vector.reciprocal(rec_pk[:], den_pk[:])
            rdram = dram.tile([H * QS], f32)
            nc.sync.dma_start(rdram[:].rearrange("(q f) -> q f", q=P), rec_pk[:])

            # normalize + assemble ctxT [128, cs, q]
            for h in range(H):
                rrep = work.tile([64, QS], f32, tag="rrep")
                src = rdram[h * QS:(h + 1) * QS]
                nc.sync.dma_start(rrep[:], bass.AP(tensor=src.tensor, offset=src.offset,
                                                   ap=[[0, 64]] + list(src.ap)))
                cn = work.tile([64, QS], bf16, tag="cn")
                nc.vector.tensor_tensor(cn[:], ctxu[:, h, :], rrep[:], ALU.mult)
                bp_ = 64 * (h % 2)
                nc.sync.dma_start(ctxT[bp_:bp_ + 64, h // 2, :], cn[:])

            # ---------- phase 7: logits_mean
            for qt in range(4):
                for kb in range(4):
                    ps = ps_misc.tile([P, 512], f32, tag="proj")
                    for cs in range(8):
                        nc.tensor.matmul(ps[:], qT[:, cs, qt * P:(qt + 1) * P],
                                         kT[:, cs, kb, :],
                                         start=(cs == 0), stop=(cs == 7))
                    lm_sb = work.tile([P, 512], f32, tag="lm")
                    nc.vector.tensor_scalar_mul(lm_sb[:], ps[:], SC_LM)
                    nc.sync.dma_start(lm_out[qt * P:(qt + 1) * P, kb * 512:(kb + 1) * 512],
                                      lm_sb[:])

            # ---------- phase 8: out-projection
            for qt in range(4):
                for ob in range(2):
                    ps = ps_misc.tile([P, 512], f32, tag="proj")
                    for cs in range(8):
                        nc.tensor.matmul(ps[:], ctxT[:, cs, qt * P:(qt + 1) * P],
                                         WpT2[:, cs, ob * 512:(ob + 1) * 512],
                                         start=(cs == 0), stop=(cs == 7))
                    x_sb = work.tile([P, 512], f32, tag="x")
                    nc.vector.tensor_tensor(x_sb[:], ps[:],
                                            bp_rep[:, ob * 512:(ob + 1) * 512],
                                            ALU.add)
                    nc.sync.dma_start(x_out[qt * P:(qt + 1) * P, ob * 512:(ob + 1) * 512],
                                      x_sb[:])

    nc.compile()
    return nc


def _get_runner():
    if "runner" in _CACHE:
        return _CACHE["runner"]
    import jax
    import concourse.mybir as mybir
    from concourse.bass2jax import _bass_exec_p, install_neuronx_cc_hook, partition_id_tensor
    from jax.sharding import Mesh, PartitionSpec, NamedSharding
    from jax.experimental.shard_map import shard_map

    nc = _build_nc()
    install_neuronx_cc_hook()
    partition_name = nc.partition_id_tensor.name if nc.partition_id_tensor else None
    in_names, out_names, out_avals, zero_shapes = [], [], [], []
    for alloc in nc.m.functions[0].allocations:
        if not isinstance(alloc, mybir.MemoryLocationSet):
            continue
        name = alloc.memorylocations[0].name
        if alloc.kind == "ExternalInput":
            if name != partition_name:
                in_names.append(name)
        elif alloc.kind == "ExternalOutput":
            out_names.append(name)
            shape = tuple(alloc.tensor_shape)
            dtype = mybir.dt.np(alloc.dtype)
            out_avals.append(jax.core.ShapedArray(shape, dtype))
            zero_shapes.append((shape, dtype))
    n_params = len(in_names)
    all_in_names = list(in_names) + list(out_names)
    if partition_name is not None:
        all_in_names.append(partition_name)

    def _body(*args):
        operands = list(args)
        if partition_name is not None:
            operands.append(partition_id_tensor())
        outs = _bass_exec_p.bind(
            *operands,
            out_avals=tuple(out_avals),
            in_names=tuple(all_in_names),
            out_names=tuple(out_names),
            lowering_input_output_aliases=(),
            sim_require_finite=True,
            sim_require_nnan=True,
            nc=nc,
        )
        return tuple(outs)

    devices = jax.devices()[:NCORES]
    mesh = Mesh(np.asarray(devices), ("core",))
    specs = (PartitionSpec("core"),)
    fn = jax.jit(
        shard_map(_body, mesh=mesh,
                  in_specs=specs * (n_params + len(out_names)),
                  out_specs=specs * len(out_names), check_rep=False),
        keep_unused=True,
    )
    sharding = NamedSharding(mesh, PartitionSpec("core"))
    _CACHE["runner"] = (fn, in_names, out_names, out_avals, zero_shapes, sharding)
    return _CACHE["runner"]


def kernel(x_q, x_kv, cls_token, Wq, bq, Wk, bk, Wv, bv, Wp, bp):
    fn, in_names, out_names, out_avals, zero_shapes, sharding = _get_runner()
    import jax

    x_q = np.asarray(x_q, dtype=np.float32)
    x_kv = np.asarray(x_kv, dtype=np.float32)
    cls_token = np.asarray(cls_token, dtype=np.float32)
    weights = {"Wq": np.asarray(Wq, np.float32), "Wk": np.asarray(Wk, np.float32),
               "Wv": np.asarray(Wv, np.float32), "Wp": np.asarray(Wp, np.float32),
               "bq": np.asarray(bq, np.float32), "bk": np.asarray(bk, np.float32),
               "bv": np.asarray(bv, np.float32), "bp": np.asarray(bp, np.float32)}

    in_maps = []
    for c in range(NCORES):
        b, r = c // 4, c % 4
        m = {"xq": np.ascontiguousarray(x_q[b, r * QS:(r + 1) * QS]),
             "xkv": np.ascontiguousarray(x_kv[b, r * KS:(r + 1) * KS]),
             "cls": np.ascontiguousarray(cls_token[b, 0])}
        m.update(weights)
        in_maps.append(m)

    concat_in = [np.concatenate([in_maps[c][nm] for c in range(NCORES)], axis=0)
                 for nm in in_names]
    concat_zeros = [np.zeros((NCORES * s[0], *s[1:]), d) for (s, d) in zero_shapes]
    dev_in = [jax.device_put(a, sharding) for a in concat_in]
    dev_zeros = [jax.device_put(a, sharding) for a in concat_zeros]
    out = fn(*dev_in, *dev_zeros)
    res = [{nm: np.asarray(out[i]).reshape(NCORES, *out_avals[i].shape)[c]
            for i, nm in enumerate(out_names)}
           for c in range(NCORES)]

    x = np.empty((B, T, C), np.float32)
    lm = np.empty((B, T, T), np.float32)
    aff = np.empty((B, 1, 1, T), np.float32)
    for c in range(NCORES):
        b, r = c // 4, c % 4
        x[b, r * QS:(r + 1) * QS] = res[c]["x_out"]
        lm[b, r * QS:(r + 1) * QS] = res[c]["lm_out"]
    aff[0, 0, 0] = res[0]["aff_out"]
    aff[1, 0, 0] = res[4]["aff_out"]
    return (x, lm, aff)
